# revision 4
# baseline (speedup 1.0000x reference)
"""Single-head attention (B=4, S=4096, D=1024, DK=128) on 8 TRN2 NeuronCores.

Sharding: core c handles batch b=c//2, query-half h=c%2 (2048 q rows).
K/V are computed pair-split: each core projects KT/VT for its own 2048
rows, then an AllGather within core pairs [[0,1],[2,3],[4,5],[6,7]]
exchanges the halves (key order = [half0, half1] on both cores of a pair,
which is softmax/AV permutation-invariant).

Engine/layout choices (contraction dims must land on SBUF partitions):
  1. x chunks -> PE transpose -> XT [d, s] (f32, rounded to f32r on the
     PSUM->SBUF drain; f32r = 11-bit-mantissa matmul mode, 4x faster than
     f32, validated end-to-end rel-err ~0.9e-2 vs 2e-2 gate).
  2. QT[dk,s] = Wq^T XT, KT[dk,s], VT[dk,s] (f32r matmuls, N=512).
  3. AllGather KT (f32) and VT (bf16) across the pair; V[s,dk] via PE
     transpose of gathered VT.
  4. Flash per 128-q block over 4 key chunks of 1024: S = QT_blk^T KT in
     PSUM (f32r), online row-max on DVE, exp+row-sum on ACT (PSUM->SBUF
     bf16 P), PE-transpose P -> PT, AV accumulates Z[q,dk] in PSUM with
     per-chunk rescale, final 1/l on the Z drain.
"""

import numpy as np

import concourse.bass as bass
import concourse.tile as tile
from concourse import bacc, mybir
from concourse.bass_utils import run_bass_kernel_spmd
from concourse.masks import make_identity

B, S, D, DK = 4, 4096, 1024, 128
SH = S // 2          # q rows per core
NCORES = 8
SCALE = 1.0 / float(np.sqrt(np.float32(DK)))

F32 = mybir.dt.float32
F32R = mybir.dt.float32r
BF16 = mybir.dt.bfloat16

N_DTILE = D // 128            # 8 contraction tiles for projections
S_CHUNK = 512                 # s-chunk for XT/projection phase
N_SCHUNK_OWN = SH // S_CHUNK  # 4
K_CHUNK = 1024                # key chunk in flash phase
N_KCHUNK = S // K_CHUNK       # 4
N_QBLK = SH // 128            # 16

SPLIT_KV = True               # pair-split K/V + AllGather vs replicated


def build_bass():
    nc = bacc.Bacc("TRN2", target_bir_lowering=False, debug=False,
                   num_devices=NCORES)
    xq = nc.dram_tensor("xq", [SH, D], F32, kind="ExternalInput").ap()
    xo = (None if SPLIT_KV else
          nc.dram_tensor("xo", [SH, D], F32, kind="ExternalInput").ap())
    wq = nc.dram_tensor("Wq", [D, DK], F32, kind="ExternalInput").ap()
    wk = nc.dram_tensor("Wk", [D, DK], F32, kind="ExternalInput").ap()
    wv = nc.dram_tensor("Wv", [D, DK], F32, kind="ExternalInput").ap()
    z = nc.dram_tensor("z", [SH, DK], F32, kind="ExternalOutput").ap()

    groups = [[0, 1], [2, 3], [4, 5], [6, 7]]

    with tile.TileContext(nc) as tc:
        with (
            tc.tile_pool(name="singles", bufs=1) as singles,
            tc.tile_pool(name="resident", bufs=1) as resident,
            tc.tile_pool(name="dram", bufs=1, space="DRAM") as dram,
        ):
            ident32 = singles.tile([128, 128], F32)
            make_identity(nc, ident32[:])
            identbf = singles.tile([128, 128], BF16)
            make_identity(nc, identbf[:])

            # weights: DRAM [D, DK] -> SBUF [128, N_DTILE, DK] f32r
            w_r = {}
            for name, w in (("wq", wq), ("wk", wk), ("wv", wv)):
                wst = singles.tile([128, N_DTILE, DK], F32, tag=f"{name}_st")
                nc.sync.dma_start(wst[:], w.rearrange("(j p) k -> p j k", p=128))
                wrt = singles.tile([128, N_DTILE, DK], F32R, tag=f"{name}_r")
                nc.any.tensor_copy(wrt[:], wst[:])
                w_r[name] = wrt

            QT = resident.tile([128, SH], F32R)      # [dk, own q rows]
            KT = resident.tile([128, S], F32R)       # [dk, all keys]
            V = resident.tile([128, S // 128, DK], BF16)  # [keys-part, t, dk]

            if SPLIT_KV:
                KT_own = resident.tile([128, SH], F32R)
                VT_own = resident.tile([128, SH], BF16)
                kt_bounce = dram.tile([128, SH], F32)
                kt_gather = dram.tile([2, 128, SH], F32)
                vt_bounce = dram.tile([128, SH], BF16)
                vt_gather = dram.tile([2, 128, SH], BF16)
            VT_full = resident.tile([128, S], BF16)

            # ---------------- Phase A/B: XT + projections ----------------
            with (
                tc.tile_pool(name="xst", bufs=10) as xst_pool,
                tc.tile_pool(name="xtsb", bufs=2) as xt_pool,
                tc.tile_pool(name="xtps", bufs=2,
                             space=bass.MemorySpace.PSUM) as xt_ps,
                tc.tile_pool(name="projps", bufs=1,
                             space=bass.MemorySpace.PSUM) as proj_ps,
            ):
                n_chunks = N_SCHUNK_OWN if SPLIT_KV else S // S_CHUNK
                for ci in range(n_chunks):
                    src = xq if ci < N_SCHUNK_OWN else xo
                    off = (ci % N_SCHUNK_OWN) * S_CHUNK

                    # load 4 x row-tiles [128, D]
                    xts = []
                    for t in range(S_CHUNK // 128):
                        xt_t = xst_pool.tile([128, D], F32, tag="xst")
                        nc.sync.dma_start(
                            xt_t[:], src[off + t * 128: off + (t + 1) * 128, :])
                        xts.append(xt_t)

                    # transpose to XT chunk [128(d), N_DTILE, S_CHUNK] f32r
                    xt_chunk = xt_pool.tile([128, N_DTILE, S_CHUNK], F32R)
                    for jd2 in range(N_DTILE // 2):
                        ps = xt_ps.tile([128, 2, S_CHUNK], F32, tag="xtps")
                        for j1 in range(2):
                            for t in range(S_CHUNK // 128):
                                nc.tensor.transpose(
                                    ps[:, j1, t * 128:(t + 1) * 128],
                                    xts[t][:, (jd2 * 2 + j1) * 128:
                                           (jd2 * 2 + j1 + 1) * 128],
                                    ident32[:])
                        nc.any.tensor_copy(
                            xt_chunk[:, jd2 * 2:(jd2 + 1) * 2, :], ps[:])

                    # projections for this chunk
                    def do_proj(wkey, dst, dst_off, dtype_drain, tag):
                        pps = proj_ps.tile([128, S_CHUNK], F32, tag=tag)
                        for jd in range(N_DTILE):
                            nc.tensor.matmul(
                                pps[:], w_r[wkey][:, jd, :], xt_chunk[:, jd, :],
                                start=(jd == 0), stop=(jd == N_DTILE - 1))
                        nc.any.tensor_copy(
                            dst[:, dst_off:dst_off + S_CHUNK], pps[:])

                    if ci < N_SCHUNK_OWN:
                        do_proj("wq", QT, ci * S_CHUNK, F32R, "q")
                    kt_dst = KT_own if SPLIT_KV else KT
                    vt_dst = VT_own if SPLIT_KV else VT_full
                    do_proj("wk", kt_dst, ci * S_CHUNK, F32R, "k")
                    do_proj("wv", vt_dst, ci * S_CHUNK, BF16, "v")

            if SPLIT_KV:
                # bounce to DRAM, AllGather within pairs, load back
                nc.sync.dma_start(kt_bounce[:], KT_own[:].bitcast(F32))
                nc.sync.dma_start(vt_bounce[:], VT_own[:])
                nc.gpsimd.collective_compute(
                    "AllGather", mybir.AluOpType.bypass,
                    replica_groups=groups,
                    ins=[kt_bounce[:].opt()], outs=[kt_gather[:].opt()])
                nc.gpsimd.collective_compute(
                    "AllGather", mybir.AluOpType.bypass,
                    replica_groups=groups,
                    ins=[vt_bounce[:].opt()], outs=[vt_gather[:].opt()])
                for r in range(2):
                    nc.sync.dma_start(
                        KT[:, r * SH:(r + 1) * SH],
                        kt_gather[r].bitcast(F32R))
                    nc.sync.dma_start(
                        VT_full[:, r * SH:(r + 1) * SH], vt_gather[r])

            # V tiles: transpose VT_full [dk, s] -> V [s, dk]
            with tc.tile_pool(name="vps", bufs=2,
                              space=bass.MemorySpace.PSUM) as v_ps:
                for cg in range(S // 1024):
                    vtp = v_ps.tile([128, 1024], BF16)
                    for t in range(8):
                        nc.tensor.transpose(
                            vtp[:, t * 128:(t + 1) * 128],
                            VT_full[:, cg * 1024 + t * 128:
                                    cg * 1024 + (t + 1) * 128],
                            identbf[:])
                    nc.any.tensor_copy(
                        V[:, cg * 8:(cg + 1) * 8, :],
                        vtp[:].rearrange("p (t k) -> p t k", k=DK))

            # ---------------- Phase C: flash attention ----------------
            with (
                tc.tile_pool(name="psb", bufs=3) as p_pool,
                tc.tile_pool(name="ptsb", bufs=3) as pt_pool,
                tc.tile_pool(name="stats", bufs=10) as stats,
                tc.tile_pool(name="zsb", bufs=2) as z_pool,
                tc.tile_pool(name="sps", bufs=2,
                             space=bass.MemorySpace.PSUM) as s_ps,
                tc.tile_pool(name="ptps", bufs=2,
                             space=bass.MemorySpace.PSUM) as pt_ps,
                tc.tile_pool(name="zps", bufs=2,
                             space=bass.MemorySpace.PSUM) as z_ps_pool,
            ):
                for qb in range(N_QBLK):
                    qsl = QT[:, qb * 128:(qb + 1) * 128]
                    z_acc = z_ps_pool.tile([128, DK], F32)
                    m_run = None
                    l_run = None
                    for j in range(N_KCHUNK):
                        sps = s_ps.tile([128, K_CHUNK], F32)
                        for half in range(K_CHUNK // 512):
                            nc.tensor.matmul(
                                sps[:, half * 512:(half + 1) * 512],
                                qsl,
                                KT[:, j * K_CHUNK + half * 512:
                                   j * K_CHUNK + (half + 1) * 512],
                                start=True, stop=True)

                        m_j = stats.tile([128, 1], F32, tag="mj")
                        nc.vector.reduce_max(
                            out=m_j[:], in_=sps[:], axis=mybir.AxisListType.X)

                        if j == 0:
                            m_new = m_j
                        else:
                            m_new = stats.tile([128, 1], F32, tag="mnew")
                            nc.vector.tensor_max(m_new[:], m_run[:], m_j[:])
                            # correction c = exp(SCALE*(m_old - m_new))
                            diff = stats.tile([128, 1], F32, tag="diff")
                            nc.vector.tensor_sub(diff[:], m_run[:], m_new[:])
                            corr = stats.tile([128, 1], F32, tag="corr")
                            nc.scalar.activation(
                                out=corr[:], in_=diff[:],
                                func=mybir.ActivationFunctionType.Exp,
                                scale=SCALE)
                            nc.vector.tensor_scalar_mul(
                                z_acc[:], z_acc[:], corr[:])

                        neg = stats.tile([128, 1], F32, tag="neg")
                        nc.vector.tensor_scalar_mul(neg[:], m_new[:], -SCALE)

                        p_sb = p_pool.tile([128, K_CHUNK], BF16)
                        l_j = stats.tile([128, 1], F32, tag="lj")
                        nc.scalar.activation(
                            out=p_sb[:], in_=sps[:],
                            func=mybir.ActivationFunctionType.Exp,
                            bias=neg[:], scale=SCALE, accum_out=l_j[:])

                        if j == 0:
                            l_new = l_j
                        else:
                            l_new = stats.tile([128, 1], F32, tag="lnew")
                            nc.vector.tensor_scalar(
                                out=l_new[:], in0=l_run[:], scalar1=corr[:],
                                scalar2=l_j[:], op0=mybir.AluOpType.mult,
                                op1=mybir.AluOpType.add)

                        # transpose P chunk -> PT [keys, q]
                        pt_sb = pt_pool.tile([128, K_CHUNK // 128, 128], BF16)
                        ptp = pt_ps.tile([128, K_CHUNK], BF16)
                        for t in range(K_CHUNK // 128):
                            nc.tensor.transpose(
                                ptp[:, t * 128:(t + 1) * 128],
                                p_sb[:, t * 128:(t + 1) * 128],
                                identbf[:])
                        nc.any.tensor_copy(
                            pt_sb[:],
                            ptp[:].rearrange("p (t k) -> p t k", k=128))

                        # AV accumulate
                        for t in range(K_CHUNK // 128):
                            nc.tensor.matmul(
                                z_acc[:],
                                pt_sb[:, t, :],
                                V[:, j * (K_CHUNK // 128) + t, :],
                                start=(j == 0 and t == 0),
                                stop=(j == N_KCHUNK - 1 and
                                      t == K_CHUNK // 128 - 1),
                                skip_group_check=True)

                        m_run = m_new
                        l_run = l_new

                    rinv = stats.tile([128, 1], F32, tag="rinv")
                    nc.vector.reciprocal(rinv[:], l_run[:])
                    z_sb = z_pool.tile([128, DK], F32)
                    nc.vector.tensor_scalar_mul(z_sb[:], z_acc[:], rinv[:])
                    nc.sync.dma_start(
                        z[qb * 128:(qb + 1) * 128, :], z_sb[:])

    nc.compile()
    return nc


_NC_CACHE = None


def _get_nc():
    global _NC_CACHE
    if _NC_CACHE is None:
        _NC_CACHE = build_bass()
    return _NC_CACHE


def make_in_maps(x, Wq, Wk, Wv):
    x = np.ascontiguousarray(np.asarray(x, dtype=np.float32))
    Wq = np.ascontiguousarray(np.asarray(Wq, dtype=np.float32))
    Wk = np.ascontiguousarray(np.asarray(Wk, dtype=np.float32))
    Wv = np.ascontiguousarray(np.asarray(Wv, dtype=np.float32))
    in_maps = []
    for c in range(NCORES):
        b, h = divmod(c, 2)
        m = {
            "xq": np.ascontiguousarray(x[b, h * SH:(h + 1) * SH]),
            "Wq": Wq, "Wk": Wk, "Wv": Wv,
        }
        if not SPLIT_KV:
            m["xo"] = np.ascontiguousarray(x[b, (1 - h) * SH:(2 - h) * SH])
        in_maps.append(m)
    return in_maps


def run(x, Wq, Wk, Wv, trace=False, **kwargs):
    nc = _get_nc()
    res = run_bass_kernel_spmd(nc, make_in_maps(x, Wq, Wk, Wv),
                               core_ids=list(range(NCORES)), trace=trace,
                               **kwargs)
    zfull = np.empty((B, S, DK), np.float32)
    for c in range(NCORES):
        b, h = divmod(c, 2)
        zfull[b, h * SH:(h + 1) * SH] = res.results[c]["z"]
    return zfull, res


def kernel(x, Wq, Wk, Wv):
    zfull, _ = run(x, Wq, Wk, Wv)
    return zfull


if __name__ == "__main__":
    rng = np.random.default_rng(0)
    x = rng.standard_normal((B, S, D), dtype=np.float32)
    Wq_ = rng.standard_normal((D, DK), dtype=np.float32)
    Wk_ = rng.standard_normal((D, DK), dtype=np.float32)
    Wv_ = rng.standard_normal((D, DK), dtype=np.float32)
    zk = kernel(x, Wq_, Wk_, Wv_)
    print("kernel output", zk.shape, zk.dtype)


# revision 5
# speedup vs baseline: 1.0835x; 1.0835x over previous
"""Single-head attention (B=4, S=4096, D=1024, DK=128) on 8 TRN2 NeuronCores.

Sharding: core c handles batch b=c//2, query-half h=c%2 (2048 q rows).
K/V are computed pair-split: each core projects KT/VT for its own 2048
rows, then an AllGather within core pairs [[0,1],[2,3],[4,5],[6,7]]
exchanges the halves (key order = [half0, half1] on both cores of a pair,
which is softmax/AV permutation-invariant).

Engine/layout choices (contraction dims must land on SBUF partitions):
  1. x chunks -> PE transpose -> XT [d, s] (f32, rounded to f32r on the
     PSUM->SBUF drain; f32r = 11-bit-mantissa matmul mode, 4x faster than
     f32, validated end-to-end rel-err ~0.9e-2 vs 2e-2 gate).
  2. QT[dk,s] = Wq^T XT, KT[dk,s], VT[dk,s] (f32r matmuls, N=512).
  3. AllGather KT (f32) and VT (bf16) across the pair; V[s,dk] via PE
     transpose of gathered VT.
  4. Flash per 128-q block over 4 key chunks of 1024: S = QT_blk^T KT in
     PSUM (f32r), online row-max on DVE, exp+row-sum on ACT (PSUM->SBUF
     bf16 P), PE-transpose P -> PT, AV accumulates Z[q,dk] in PSUM with
     per-chunk rescale, final 1/l on the Z drain.
"""

import numpy as np

import concourse.bass as bass
import concourse.tile as tile
from concourse import bacc, mybir
from concourse.bass_utils import run_bass_kernel_spmd
from concourse.masks import make_identity

B, S, D, DK = 4, 4096, 1024, 128
SH = S // 2          # q rows per core
NCORES = 8
SCALE = 1.0 / float(np.sqrt(np.float32(DK)))

F32 = mybir.dt.float32
F32R = mybir.dt.float32r
BF16 = mybir.dt.bfloat16

N_DTILE = D // 128            # 8 contraction tiles for projections
S_CHUNK = 512                 # s-chunk for XT/projection phase
N_SCHUNK_OWN = SH // S_CHUNK  # 4
K_CHUNK = 1024                # key chunk in flash phase
N_KCHUNK = S // K_CHUNK       # 4
N_QBLK = SH // 128            # 16

SPLIT_KV = False               # pair-split K/V + AllGather vs replicated


def build_bass():
    nc = bacc.Bacc("TRN2", target_bir_lowering=False, debug=False,
                   num_devices=NCORES)
    xq = nc.dram_tensor("xq", [SH, D], F32, kind="ExternalInput").ap()
    xo = (None if SPLIT_KV else
          nc.dram_tensor("xo", [SH, D], F32, kind="ExternalInput").ap())
    wq = nc.dram_tensor("Wq", [D, DK], F32, kind="ExternalInput").ap()
    wk = nc.dram_tensor("Wk", [D, DK], F32, kind="ExternalInput").ap()
    wv = nc.dram_tensor("Wv", [D, DK], F32, kind="ExternalInput").ap()
    z = nc.dram_tensor("z", [SH, DK], F32, kind="ExternalOutput").ap()

    groups = [[0, 1], [2, 3], [4, 5], [6, 7]]

    with tile.TileContext(nc) as tc:
        with (
            tc.tile_pool(name="singles", bufs=1) as singles,
            tc.tile_pool(name="resident", bufs=1) as resident,
            tc.tile_pool(name="dram", bufs=1, space="DRAM") as dram,
        ):
            ident32 = singles.tile([128, 128], F32)
            make_identity(nc, ident32[:])
            identbf = singles.tile([128, 128], BF16)
            make_identity(nc, identbf[:])

            # weights: DRAM [D, DK] -> SBUF [128, N_DTILE, DK] f32r
            w_r = {}
            for name, w in (("wq", wq), ("wk", wk), ("wv", wv)):
                wst = singles.tile([128, N_DTILE, DK], F32, tag=f"{name}_st")
                nc.sync.dma_start(wst[:], w.rearrange("(j p) k -> p j k", p=128))
                wrt = singles.tile([128, N_DTILE, DK], F32R, tag=f"{name}_r")
                nc.any.tensor_copy(wrt[:], wst[:])
                w_r[name] = wrt

            QT = resident.tile([128, SH], F32R)      # [dk, own q rows]
            KT = resident.tile([128, S], F32R)       # [dk, all keys]
            V = resident.tile([128, S // 128, DK], BF16)  # [keys-part, t, dk]

            if SPLIT_KV:
                KT_own = resident.tile([128, SH], F32R)
                VT_own = resident.tile([128, SH], BF16)
                kt_bounce = dram.tile([128, SH], F32)
                kt_gather = dram.tile([2, 128, SH], F32)
                vt_bounce = dram.tile([128, SH], BF16)
                vt_gather = dram.tile([2, 128, SH], BF16)
            VT_full = resident.tile([128, S], BF16)

            # ---------------- Phase A/B: XT + projections ----------------
            with (
                tc.tile_pool(name="xst", bufs=10) as xst_pool,
                tc.tile_pool(name="xtsb", bufs=2) as xt_pool,
                tc.tile_pool(name="xtps", bufs=2,
                             space=bass.MemorySpace.PSUM) as xt_ps,
                tc.tile_pool(name="projps", bufs=1,
                             space=bass.MemorySpace.PSUM) as proj_ps,
            ):
                n_chunks = N_SCHUNK_OWN if SPLIT_KV else S // S_CHUNK
                for ci in range(n_chunks):
                    src = xq if ci < N_SCHUNK_OWN else xo
                    off = (ci % N_SCHUNK_OWN) * S_CHUNK

                    # load 4 x row-tiles [128, D]
                    xts = []
                    for t in range(S_CHUNK // 128):
                        xt_t = xst_pool.tile([128, D], F32, tag="xst")
                        nc.sync.dma_start(
                            xt_t[:], src[off + t * 128: off + (t + 1) * 128, :])
                        xts.append(xt_t)

                    # transpose to XT chunk [128(d), N_DTILE, S_CHUNK] f32r
                    xt_chunk = xt_pool.tile([128, N_DTILE, S_CHUNK], F32R)
                    for jd2 in range(N_DTILE // 2):
                        ps = xt_ps.tile([128, 2, S_CHUNK], F32, tag="xtps")
                        for j1 in range(2):
                            for t in range(S_CHUNK // 128):
                                nc.tensor.transpose(
                                    ps[:, j1, t * 128:(t + 1) * 128],
                                    xts[t][:, (jd2 * 2 + j1) * 128:
                                           (jd2 * 2 + j1 + 1) * 128],
                                    ident32[:])
                        nc.any.tensor_copy(
                            xt_chunk[:, jd2 * 2:(jd2 + 1) * 2, :], ps[:])

                    # projections for this chunk
                    def do_proj(wkey, dst, dst_off, dtype_drain, tag):
                        pps = proj_ps.tile([128, S_CHUNK], F32, tag=tag)
                        for jd in range(N_DTILE):
                            nc.tensor.matmul(
                                pps[:], w_r[wkey][:, jd, :], xt_chunk[:, jd, :],
                                start=(jd == 0), stop=(jd == N_DTILE - 1))
                        nc.any.tensor_copy(
                            dst[:, dst_off:dst_off + S_CHUNK], pps[:])

                    if ci < N_SCHUNK_OWN:
                        do_proj("wq", QT, ci * S_CHUNK, F32R, "q")
                    kt_dst = KT_own if SPLIT_KV else KT
                    vt_dst = VT_own if SPLIT_KV else VT_full
                    do_proj("wk", kt_dst, ci * S_CHUNK, F32R, "k")
                    do_proj("wv", vt_dst, ci * S_CHUNK, BF16, "v")

            if SPLIT_KV:
                # bounce to DRAM, AllGather within pairs, load back
                nc.sync.dma_start(kt_bounce[:], KT_own[:].bitcast(F32))
                nc.sync.dma_start(vt_bounce[:], VT_own[:])
                nc.gpsimd.collective_compute(
                    "AllGather", mybir.AluOpType.bypass,
                    replica_groups=groups,
                    ins=[kt_bounce[:].opt()], outs=[kt_gather[:].opt()])
                nc.gpsimd.collective_compute(
                    "AllGather", mybir.AluOpType.bypass,
                    replica_groups=groups,
                    ins=[vt_bounce[:].opt()], outs=[vt_gather[:].opt()])
                for r in range(2):
                    nc.sync.dma_start(
                        KT[:, r * SH:(r + 1) * SH],
                        kt_gather[r].bitcast(F32R))
                    nc.sync.dma_start(
                        VT_full[:, r * SH:(r + 1) * SH], vt_gather[r])

            # V tiles: transpose VT_full [dk, s] -> V [s, dk]
            with tc.tile_pool(name="vps", bufs=2,
                              space=bass.MemorySpace.PSUM) as v_ps:
                for cg in range(S // 1024):
                    vtp = v_ps.tile([128, 1024], BF16)
                    for t in range(8):
                        nc.tensor.transpose(
                            vtp[:, t * 128:(t + 1) * 128],
                            VT_full[:, cg * 1024 + t * 128:
                                    cg * 1024 + (t + 1) * 128],
                            identbf[:])
                    nc.any.tensor_copy(
                        V[:, cg * 8:(cg + 1) * 8, :],
                        vtp[:].rearrange("p (t k) -> p t k", k=DK))

            # ---------------- Phase C: flash attention ----------------
            with (
                tc.tile_pool(name="psb", bufs=3) as p_pool,
                tc.tile_pool(name="ptsb", bufs=3) as pt_pool,
                tc.tile_pool(name="stats", bufs=10) as stats,
                tc.tile_pool(name="zsb", bufs=2) as z_pool,
                tc.tile_pool(name="sps", bufs=2,
                             space=bass.MemorySpace.PSUM) as s_ps,
                tc.tile_pool(name="ptps", bufs=2,
                             space=bass.MemorySpace.PSUM) as pt_ps,
                tc.tile_pool(name="zps", bufs=2,
                             space=bass.MemorySpace.PSUM) as z_ps_pool,
            ):
                for qb in range(N_QBLK):
                    qsl = QT[:, qb * 128:(qb + 1) * 128]
                    z_acc = z_ps_pool.tile([128, DK], F32)
                    m_run = None
                    l_run = None
                    for j in range(N_KCHUNK):
                        sps = s_ps.tile([128, K_CHUNK], F32)
                        for half in range(K_CHUNK // 512):
                            nc.tensor.matmul(
                                sps[:, half * 512:(half + 1) * 512],
                                qsl,
                                KT[:, j * K_CHUNK + half * 512:
                                   j * K_CHUNK + (half + 1) * 512],
                                start=True, stop=True)

                        m_j = stats.tile([128, 1], F32, tag="mj")
                        nc.vector.reduce_max(
                            out=m_j[:], in_=sps[:], axis=mybir.AxisListType.X)

                        if j == 0:
                            m_new = m_j
                        else:
                            m_new = stats.tile([128, 1], F32, tag="mnew")
                            nc.vector.tensor_max(m_new[:], m_run[:], m_j[:])
                            # correction c = exp(SCALE*(m_old - m_new))
                            diff = stats.tile([128, 1], F32, tag="diff")
                            nc.vector.tensor_sub(diff[:], m_run[:], m_new[:])
                            corr = stats.tile([128, 1], F32, tag="corr")
                            nc.scalar.activation(
                                out=corr[:], in_=diff[:],
                                func=mybir.ActivationFunctionType.Exp,
                                scale=SCALE)
                            nc.vector.tensor_scalar_mul(
                                z_acc[:], z_acc[:], corr[:])

                        neg = stats.tile([128, 1], F32, tag="neg")
                        nc.vector.tensor_scalar_mul(neg[:], m_new[:], -SCALE)

                        p_sb = p_pool.tile([128, K_CHUNK], BF16)
                        l_j = stats.tile([128, 1], F32, tag="lj")
                        nc.scalar.activation(
                            out=p_sb[:], in_=sps[:],
                            func=mybir.ActivationFunctionType.Exp,
                            bias=neg[:], scale=SCALE, accum_out=l_j[:])

                        if j == 0:
                            l_new = l_j
                        else:
                            l_new = stats.tile([128, 1], F32, tag="lnew")
                            nc.vector.tensor_scalar(
                                out=l_new[:], in0=l_run[:], scalar1=corr[:],
                                scalar2=l_j[:], op0=mybir.AluOpType.mult,
                                op1=mybir.AluOpType.add)

                        # transpose P chunk -> PT [keys, q]
                        pt_sb = pt_pool.tile([128, K_CHUNK // 128, 128], BF16)
                        ptp = pt_ps.tile([128, K_CHUNK], BF16)
                        for t in range(K_CHUNK // 128):
                            nc.tensor.transpose(
                                ptp[:, t * 128:(t + 1) * 128],
                                p_sb[:, t * 128:(t + 1) * 128],
                                identbf[:])
                        nc.any.tensor_copy(
                            pt_sb[:],
                            ptp[:].rearrange("p (t k) -> p t k", k=128))

                        # AV accumulate
                        for t in range(K_CHUNK // 128):
                            nc.tensor.matmul(
                                z_acc[:],
                                pt_sb[:, t, :],
                                V[:, j * (K_CHUNK // 128) + t, :],
                                start=(j == 0 and t == 0),
                                stop=(j == N_KCHUNK - 1 and
                                      t == K_CHUNK // 128 - 1),
                                skip_group_check=True)

                        m_run = m_new
                        l_run = l_new

                    rinv = stats.tile([128, 1], F32, tag="rinv")
                    nc.vector.reciprocal(rinv[:], l_run[:])
                    z_sb = z_pool.tile([128, DK], F32)
                    nc.vector.tensor_scalar_mul(z_sb[:], z_acc[:], rinv[:])
                    nc.sync.dma_start(
                        z[qb * 128:(qb + 1) * 128, :], z_sb[:])

    nc.compile()
    return nc


_NC_CACHE = None


def _get_nc():
    global _NC_CACHE
    if _NC_CACHE is None:
        _NC_CACHE = build_bass()
    return _NC_CACHE


def make_in_maps(x, Wq, Wk, Wv):
    x = np.ascontiguousarray(np.asarray(x, dtype=np.float32))
    Wq = np.ascontiguousarray(np.asarray(Wq, dtype=np.float32))
    Wk = np.ascontiguousarray(np.asarray(Wk, dtype=np.float32))
    Wv = np.ascontiguousarray(np.asarray(Wv, dtype=np.float32))
    in_maps = []
    for c in range(NCORES):
        b, h = divmod(c, 2)
        m = {
            "xq": np.ascontiguousarray(x[b, h * SH:(h + 1) * SH]),
            "Wq": Wq, "Wk": Wk, "Wv": Wv,
        }
        if not SPLIT_KV:
            m["xo"] = np.ascontiguousarray(x[b, (1 - h) * SH:(2 - h) * SH])
        in_maps.append(m)
    return in_maps


def run(x, Wq, Wk, Wv, trace=False, **kwargs):
    nc = _get_nc()
    res = run_bass_kernel_spmd(nc, make_in_maps(x, Wq, Wk, Wv),
                               core_ids=list(range(NCORES)), trace=trace,
                               **kwargs)
    zfull = np.empty((B, S, DK), np.float32)
    for c in range(NCORES):
        b, h = divmod(c, 2)
        zfull[b, h * SH:(h + 1) * SH] = res.results[c]["z"]
    return zfull, res


def kernel(x, Wq, Wk, Wv):
    zfull, _ = run(x, Wq, Wk, Wv)
    return zfull


if __name__ == "__main__":
    rng = np.random.default_rng(0)
    x = rng.standard_normal((B, S, D), dtype=np.float32)
    Wq_ = rng.standard_normal((D, DK), dtype=np.float32)
    Wk_ = rng.standard_normal((D, DK), dtype=np.float32)
    Wv_ = rng.standard_normal((D, DK), dtype=np.float32)
    zk = kernel(x, Wq_, Wk_, Wv_)
    print("kernel output", zk.shape, zk.dtype)
